# revision 7
# baseline (speedup 1.0000x reference)
"""Trainium2 Bass kernel for nn_BaselineTrustModel.

Math (see the reference): the per-timestep recurrence is affine and collapses
to a per-sample scalar formula.  With
    s    = sum_t perf[t, n]                (number of "fail" flags, 0..T)
    mask = any(obs[0, n, :] != 0)
    r1   = 1/sqrt(sigma0^2 + T*sigma_t^2)
    z0   = trust0/sqrt(sigma0^2)
    A    = (trust0 + T*wb + T*wtp) * r1
    B    = 2*wtp*r1
the output is
    pred[n] = clip(sigmoid(z0 + mask*( (A - z0) - B*s )), 0.01, 0.99)

Only obs[0] (N x D) and perf (T x N) are ever read -> ~66 MB of f32 input
traffic total, data-parallel over the sample axis N across 8 cores.

Device kernel per core (Nc = N/8 samples, padded to K*128*F):
  per tile k:  DMA obs0[k] (128 x F*16) and perfT[k] (128 x F*16) to SBUF,
  DVE: s = reduce_add(perf, last dim), ma = reduce_absmax(obs, last dim),
       d = s*(-B) + (A-z0),  x = (ma > 0) * d
  ACT: p = sigmoid(x + z0)
  DVE: o = clip(p, 0.01, 0.99)   -> DMA out (128 x F)
"""

import math
import sys

import numpy as np

for _p in ("/opt/trn_rl_repo", "/root/.axon_site/_ro/trn_rl_repo"):
    if _p not in sys.path:
        sys.path.append(_p)

T = 16
D = 16
N = 500000
NCORES = 8

# per-core tiling: K tiles of 128 partitions x F samples
F = 49
K = 10
PER = K * 128 * F          # 62720 samples per core
NPAD = NCORES * PER        # 501760 total (1760 zero-padded samples)


def build_program(ktiles, fsz, neg_b, c_const, z0):
    """Build the single-core Bass/Tile program (SPMD across cores)."""
    import concourse.tile as tile
    from concourse import bacc, mybir

    f32 = mybir.dt.float32
    nc = bacc.Bacc("TRN2", target_bir_lowering=False, debug=False)
    obs_d = nc.dram_tensor("obs0", [ktiles, 128, fsz * D], f32, kind="ExternalInput").ap()
    perf_d = nc.dram_tensor("perfT", [ktiles, 128, fsz * T], f32, kind="ExternalInput").ap()
    out_d = nc.dram_tensor("out", [ktiles, 128, fsz], f32, kind="ExternalOutput").ap()

    with tile.TileContext(nc) as tc:
        with (
            tc.tile_pool(name="const", bufs=1) as cpool,
            tc.tile_pool(name="io", bufs=4) as iop,
            tc.tile_pool(name="sm", bufs=4) as smp,
        ):
            z0t = cpool.tile([128, 1], f32, tag="z0")
            nc.vector.memset(z0t[:], z0)
            for k in range(ktiles):
                to = iop.tile([128, fsz * D], f32, tag="obs")
                nc.sync.dma_start(to[:], obs_d[k])
                tp = iop.tile([128, fsz * T], f32, tag="perf")
                nc.sync.dma_start(tp[:], perf_d[k])

                s = smp.tile([128, fsz], f32, tag="s")
                nc.vector.tensor_reduce(
                    s[:],
                    tp[:].rearrange("p (f d) -> p f d", d=T),
                    axis=mybir.AxisListType.X,
                    op=mybir.AluOpType.add,
                )
                ma = smp.tile([128, fsz], f32, tag="ma")
                nc.vector.tensor_reduce(
                    ma[:],
                    to[:].rearrange("p (f d) -> p f d", d=D),
                    axis=mybir.AxisListType.X,
                    op=mybir.AluOpType.max,
                    apply_absolute_value=True,
                )
                dd = smp.tile([128, fsz], f32, tag="dd")
                nc.vector.tensor_scalar(
                    dd[:], s[:], neg_b, c_const,
                    op0=mybir.AluOpType.mult, op1=mybir.AluOpType.add,
                )
                x = smp.tile([128, fsz], f32, tag="x")
                nc.vector.scalar_tensor_tensor(
                    x[:], ma[:], 0.0, dd[:],
                    op0=mybir.AluOpType.is_gt, op1=mybir.AluOpType.mult,
                )
                p = smp.tile([128, fsz], f32, tag="p")
                nc.scalar.activation(
                    p[:], x[:], mybir.ActivationFunctionType.Sigmoid,
                    bias=z0t[:], scale=1.0,
                )
                o = smp.tile([128, fsz], f32, tag="o")
                nc.vector.tensor_scalar(
                    o[:], p[:], 0.01, 0.99,
                    op0=mybir.AluOpType.max, op1=mybir.AluOpType.min,
                )
                nc.sync.dma_start(out_d[k], o[:])
    nc.compile()
    return nc


def _scalar_constants(inputs):
    t0 = float(np.asarray(inputs["trust0"]).reshape(()))
    s0 = float(np.asarray(inputs["sigma0"]).reshape(()))
    wb = float(np.asarray(inputs["wb"]).reshape(()))
    wtp = float(np.asarray(inputs["wtp"]).reshape(()))
    st = float(np.asarray(inputs["sigma_t"]).reshape(()))
    r1 = 1.0 / math.sqrt(s0 * s0 + T * st * st)
    z0 = t0 / math.sqrt(s0 * s0)
    a_const = (t0 + T * wb + T * wtp) * r1
    neg_b = -2.0 * wtp * r1
    c_const = a_const - z0
    return neg_b, c_const, z0


def run(inputs, trace=False, **kw):
    """Shard, run on 8 cores, gather. Returns (output [N,1] f32, exec_time_ns)."""
    from concourse.bass_utils import run_bass_kernel_spmd

    obs = np.asarray(inputs["inptasksobs"])
    perf = np.asarray(inputs["inptasksperf"])
    assert obs.shape == (T, N, D) and perf.shape == (T, N, 1)

    neg_b, c_const, z0 = _scalar_constants(inputs)
    nc = build_program(K, F, neg_b, c_const, z0)

    obs_p = np.zeros((NPAD, D), np.float32)
    obs_p[:N] = obs[0]
    perf_p = np.zeros((NPAD, T), np.float32)
    perf_p[:N] = perf[:, :, 0].T

    obs_sh = obs_p.reshape(NCORES, K, 128, F * D)
    perf_sh = perf_p.reshape(NCORES, K, 128, F * T)
    in_maps = [
        {"obs0": obs_sh[c], "perfT": perf_sh[c]} for c in range(NCORES)
    ]

    res = run_bass_kernel_spmd(
        nc, in_maps, core_ids=list(range(NCORES)), trace=trace, **kw
    )
    full = np.concatenate(
        [res.results[c]["out"].reshape(-1) for c in range(NCORES)]
    )
    return full[:N].reshape(N, 1).astype(np.float32, copy=False), res.exec_time_ns


def kernel(**inputs):
    out, _ = run(inputs, trace=False)
    return out


# revision 8
# speedup vs baseline: 1.1677x; 1.1677x over previous
"""Trainium2 Bass kernel for nn_BaselineTrustModel.

Math (see the reference): the per-timestep recurrence is affine and collapses
to a per-sample scalar formula.  With
    s    = sum_t perf[t, n]                (number of "fail" flags, 0..T)
    mask = any(obs[0, n, :] != 0)
    r1   = 1/sqrt(sigma0^2 + T*sigma_t^2)
    z0   = trust0/sqrt(sigma0^2)
    A    = (trust0 + T*wb + T*wtp) * r1
    B    = 2*wtp*r1
the output is
    pred[n] = clip(sigmoid(z0 + mask*( (A - z0) - B*s )), 0.01, 0.99)

Only obs[0] (N x D) and perf (T x N) are ever read -> ~66 MB of f32 input
traffic total, data-parallel over the sample axis N across 8 cores
(~8.3 MB per core, memory-bound).

Device kernel per core (Nc = K128*F samples, partition p owns samples
[p*F, (p+1)*F) of its core shard):
  - perf T-sum via DMA inline compute: 16 accumulate-DMAs (SWDGE cce add)
    into NPART partial tiles [128, F], combined with NPART-1 DVE adds.
    The vector engine never touches the 4 MB of perf data.
  - obs[0] loaded in K chunks [128, (F/K)*16]; DVE segmented abs-max reduce
    per chunk -> mask.
  - single fused epilogue on [128, F]: d = s*(-B) + (A-z0);
    x = (ma > 0) * d;  p = sigmoid(x + z0) on ACT;  clip; one store.
"""

import math
import sys

import numpy as np

for _p in ("/opt/trn_rl_repo", "/root/.axon_site/_ro/trn_rl_repo"):
    if _p not in sys.path:
        sys.path.append(_p)

T = 16
D = 16
N = 500000
NCORES = 8

F = 490            # samples per partition per core
K = 5              # obs load/reduce chunks (F % K == 0)
NPART = 4          # parallel perf-accumulation chains (T % NPART == 0)
PER = 128 * F      # 62720 samples per core
NPAD = NCORES * PER


def build_program(fsz, kchunks, npart, neg_b, c_const, z0):
    """Build the single-core Bass/Tile program (SPMD across cores)."""
    import concourse.tile as tile
    from concourse import bacc, mybir

    f32 = mybir.dt.float32
    fc = fsz // kchunks
    nc = bacc.Bacc("TRN2", target_bir_lowering=False, debug=False)
    obs_d = nc.dram_tensor("obs0", [128, kchunks, fc * D], f32, kind="ExternalInput").ap()
    perf_d = nc.dram_tensor("perfc", [T, 128, fsz], f32, kind="ExternalInput").ap()
    out_d = nc.dram_tensor("out", [128, fsz], f32, kind="ExternalOutput").ap()

    with tile.TileContext(nc) as tc:
        with (
            tc.tile_pool(name="stats", bufs=1) as spool,
            tc.tile_pool(name="io", bufs=3) as iop,
            tc.tile_pool(name="ep", bufs=1) as epool,
        ):
            z0t = spool.tile([128, 1], f32, tag="z0")
            nc.vector.memset(z0t[:], z0)

            # perf: T-sum computed by the DMA engines (cce accumulate),
            # npart parallel chains of T/npart serialized links each.
            sparts = []
            for j in range(npart):
                sp = spool.tile([128, fsz], f32, tag=f"sp{j}")
                nc.gpsimd.memset(sp[:], 0.0)
                sparts.append(sp)
            for t in range(T):
                nc.gpsimd.dma_start(
                    sparts[t % npart][:], perf_d[t],
                    accum_op=mybir.AluOpType.add,
                )

            # obs: chunked load + segmented abs-max reduce on DVE
            ma = spool.tile([128, fsz], f32, tag="ma")
            for k in range(kchunks):
                to = iop.tile([128, fc * D], f32, tag="obs")
                nc.sync.dma_start(to[:], obs_d[:, k])
                nc.vector.tensor_reduce(
                    ma[:, k * fc:(k + 1) * fc],
                    to[:].rearrange("p (f d) -> p f d", d=D),
                    axis=mybir.AxisListType.X,
                    op=mybir.AluOpType.max,
                    apply_absolute_value=True,
                )

            # combine partial perf sums: s = ((sp0+sp1)+(sp2+sp3))...
            s = sparts[0]
            for j in range(1, npart):
                acc = epool.tile([128, fsz], f32, tag=f"acc{j}")
                nc.vector.tensor_tensor(
                    acc[:], s[:], sparts[j][:], op=mybir.AluOpType.add
                )
                s = acc

            # epilogue (one pass over [128, fsz])
            dd = epool.tile([128, fsz], f32, tag="dd")
            nc.vector.tensor_scalar(
                dd[:], s[:], neg_b, c_const,
                op0=mybir.AluOpType.mult, op1=mybir.AluOpType.add,
            )
            x = epool.tile([128, fsz], f32, tag="x")
            nc.vector.scalar_tensor_tensor(
                x[:], ma[:], 0.0, dd[:],
                op0=mybir.AluOpType.is_gt, op1=mybir.AluOpType.mult,
            )
            p = epool.tile([128, fsz], f32, tag="p")
            nc.scalar.activation(
                p[:], x[:], mybir.ActivationFunctionType.Sigmoid,
                bias=z0t[:], scale=1.0,
            )
            o = epool.tile([128, fsz], f32, tag="o")
            nc.vector.tensor_scalar(
                o[:], p[:], 0.01, 0.99,
                op0=mybir.AluOpType.max, op1=mybir.AluOpType.min,
            )
            nc.sync.dma_start(out_d[:], o[:])
    nc.compile()
    return nc


def _scalar_constants(inputs):
    t0 = float(np.asarray(inputs["trust0"]).reshape(()))
    s0 = float(np.asarray(inputs["sigma0"]).reshape(()))
    wb = float(np.asarray(inputs["wb"]).reshape(()))
    wtp = float(np.asarray(inputs["wtp"]).reshape(()))
    st = float(np.asarray(inputs["sigma_t"]).reshape(()))
    r1 = 1.0 / math.sqrt(s0 * s0 + T * st * st)
    z0 = t0 / math.sqrt(s0 * s0)
    a_const = (t0 + T * wb + T * wtp) * r1
    neg_b = -2.0 * wtp * r1
    c_const = a_const - z0
    return neg_b, c_const, z0


def run(inputs, trace=False, **kw):
    """Shard, run on 8 cores, gather. Returns (output [N,1] f32, exec_time_ns)."""
    from concourse.bass_utils import run_bass_kernel_spmd

    obs = np.asarray(inputs["inptasksobs"])
    perf = np.asarray(inputs["inptasksperf"])
    assert obs.shape == (T, N, D) and perf.shape == (T, N, 1)

    neg_b, c_const, z0 = _scalar_constants(inputs)
    nc = build_program(F, K, NPART, neg_b, c_const, z0)

    obs_p = np.zeros((NPAD, D), np.float32)
    obs_p[:N] = obs[0]
    perf_p = np.zeros((T, NPAD), np.float32)
    perf_p[:, :N] = perf[:, :, 0]

    in_maps = []
    for c in range(NCORES):
        oc = obs_p[c * PER:(c + 1) * PER].reshape(128, K, (F // K) * D)
        pc = np.ascontiguousarray(
            perf_p[:, c * PER:(c + 1) * PER]
        ).reshape(T, 128, F)
        in_maps.append({"obs0": oc, "perfc": pc})

    res = run_bass_kernel_spmd(
        nc, in_maps, core_ids=list(range(NCORES)), trace=trace, **kw
    )
    full = np.concatenate(
        [res.results[c]["out"].reshape(-1) for c in range(NCORES)]
    )
    return full[:N].reshape(N, 1).astype(np.float32, copy=False), res.exec_time_ns


def kernel(**inputs):
    out, _ = run(inputs, trace=False)
    return out


# revision 40
# speedup vs baseline: 1.3181x; 1.1288x over previous
"""Trainium2 Bass kernel for nn_BaselineTrustModel.

Math (see the reference): the per-timestep recurrence is affine and collapses
to a per-sample scalar formula.  With
    s    = sum_t perf[t, n]                (number of "fail" flags, 0..T)
    mask = any(obs[0, n, :] != 0)
    r1   = 1/sqrt(sigma0^2 + T*sigma_t^2)
    z0   = trust0/sqrt(sigma0^2)
    A    = (trust0 + T*wb + T*wtp) * r1
    B    = 2*wtp*r1
the output is
    pred[n] = clip(sigmoid(z0 + mask*( (A - z0) - B*s )), 0.01, 0.99)

Only obs[0] (N x D) and perf (T x N) are ever read -> ~66 MB of f32 input
traffic total, data-parallel over the sample axis N across 8 cores
(~8.3 MB per core, memory-bound; per-core HBM roofline ~358 GB/s -> ~23 us).

Device kernel per core (raw bacc, hand-scheduled; no TileContext so we skip
its ~10 us kernel-tail barrier butterfly).  Partition p owns samples
[p*F, (p+1)*F) of the core shard (F = 490).
  SP  : 8 perf chunk loads [4t x 128 x 245m] (HWDGE), 2 result stores
  ACT : 5 obs chunk loads [128 x 98*16] on its own HWDGE queue, 2 sigmoids
  DVE : perf chunk folds (tree adds), segmented abs-max obs reduces,
        epilogue d = s*(-B)+(A-z0); x = (ma>0)*d; clip - split in two
        m-halves so the first store overlaps the second half's compute
"""

import math
import sys
from contextlib import ExitStack

import numpy as np

for _p in ("/opt/trn_rl_repo", "/root/.axon_site/_ro/trn_rl_repo"):
    if _p not in sys.path:
        sys.path.append(_p)

T = 16
D = 16
N = 500000
NCORES = 8

F = 490            # samples per partition per core
K = 5              # obs load/reduce chunks (F % K == 0)
MH = F // 2        # 245: epilogue half-width
TQ = 4             # t-layers per perf chunk
PER = 128 * F      # 62720 samples per core
NPAD = NCORES * PER


def build_program(neg_b, c_const, z0):
    """Raw-bacc single-core program (SPMD across cores)."""
    from concourse import bacc, mybir

    f32 = mybir.dt.float32
    fc = F // K                      # 98 samples per obs chunk per partition
    nchunks = (T // TQ) * 2          # 8 perf chunks (t-quarter x m-half)
    nc = bacc.Bacc("TRN2", target_bir_lowering=False, debug=False)
    obs_d = nc.dram_tensor("obs0", [128, K, fc * D], f32, kind="ExternalInput").ap()
    perf_d = nc.dram_tensor("perfc", [T, 128, F], f32, kind="ExternalInput").ap()
    out_d = nc.dram_tensor("out", [128, F], f32, kind="ExternalOutput").ap()

    with ExitStack() as ctx:
        sb = lambda name, shape: ctx.enter_context(nc.sbuf_tensor(name, shape, f32))
        pb = [sb(f"pb{j}", [128, TQ * MH]) for j in range(3)]   # perf chunk bufs
        ob = [sb(f"ob{j}", [128, fc * D]) for j in range(3)]    # obs chunk bufs
        qa = sb("qa", [128, MH])                                # fold scratch
        qb = sb("qb", [128, MH])
        qs = [sb(f"q{c}", [128, MH]) for c in range(nchunks)]   # chunk sums
        sh = [sb(f"s{h}", [128, MH]) for h in range(2)]
        s2 = [sb(f"s2{h}", [128, MH]) for h in range(2)]
        ma = sb("ma", [128, F])
        dd = [sb(f"dd{h}", [128, MH]) for h in range(2)]
        xx = [sb(f"xx{h}", [128, MH]) for h in range(2)]
        pp = [sb(f"pp{h}", [128, MH]) for h in range(2)]
        oo = [sb(f"oo{h}", [128, MH]) for h in range(2)]
        z0t = sb("z0t", [128, 1])

        pdma = [ctx.enter_context(nc.semaphore(f"pdma{j}")) for j in range(3)]
        obdma = [ctx.enter_context(nc.semaphore(f"obdma{j}")) for j in range(3)]
        odma = ctx.enter_context(nc.semaphore("odma"))
        dve = ctx.enter_context(nc.semaphore("dve"))
        act = ctx.enter_context(nc.semaphore("act"))
        all_sems = pdma + obdma + [odma, dve, act]
        nums = sorted(s.num for s in all_sems)
        assert nums == list(range(nums[0], nums[0] + len(nums))), nums
        sem_range = range(nums[0], nums[-1] + 1)
        block_cm = nc.Block()
        block = block_cm.__enter__()

        # ---- static DVE-semaphore schedule -------------------------------
        # DVE op order (counter value AFTER each op):
        #  1: memset z0t
        #  folds f_c (3 ops each), obs reduces r_k (1 op), interleaved:
        #  f0 f1 r0 f2 r1 f3 r2 | comb0(3) d0 x0 | f4 r3 f5 r4 f6 f7 |
        #  comb1(3) d1 x1 | clip0 clip1
        n = 1
        qdone, rdone = {}, {}
        order = ["f0", "f1", "r0", "f2", "r1", "f3", "r2",
                 "c0", "e0", "f4", "r3", "f5", "r4", "f6", "f7",
                 "c1", "e1"]
        for tok in order:
            if tok[0] == "f":
                n += 3
                qdone[int(tok[1])] = n
            elif tok[0] == "r":
                n += 1
                rdone[int(tok[1])] = n
            elif tok[0] == "c":
                n += 3
            elif tok[0] == "e":
                n += 2
                if tok == "e0":
                    x0_n = n
                else:
                    x1_n = n
        clip0_n, clip1_n = n + 1, n + 2

        def perf_view(c):
            h, tq = divmod(c, T // TQ)
            return perf_d[tq * TQ:(tq + 1) * TQ, :, h * MH:(h + 1) * MH]

        @block.sync
        def _(sync):
            for c in range(nchunks):
                if c >= 3:
                    sync.wait_ge(dve, qdone[c - 3])
                sync.dma_start(
                    pb[c % 3][:],
                    perf_view(c).rearrange("t p m -> p t m"),
                ).then_inc(pdma[c % 3], 16)
            sync.wait_ge(dve, clip0_n)
            sync.dma_start(out_d[:, 0:MH], oo[0][:]).then_inc(odma, 16)
            sync.wait_ge(dve, clip1_n)
            sync.dma_start(out_d[:, MH:F], oo[1][:]).then_inc(odma, 16)
            sync.wait_ge(odma, 32)

        @block.scalar
        def _(scalar):
            for k in range(K):
                if k >= 3:
                    scalar.wait_ge(dve, rdone[k - 3])
                scalar.dma_start(ob[k % 3][:], obs_d[:, k]).then_inc(obdma[k % 3], 16)
            scalar.wait_ge(dve, x0_n)
            nc.scalar.activation(
                pp[0][:], xx[0][:], mybir.ActivationFunctionType.Sigmoid,
                bias=z0t[:], scale=1.0,
            ).then_inc(act, 1)
            scalar.wait_ge(dve, x1_n)
            nc.scalar.activation(
                pp[1][:], xx[1][:], mybir.ActivationFunctionType.Sigmoid,
                bias=z0t[:], scale=1.0,
            ).then_inc(act, 1)

        @block.vector
        def _(vector):
            # running count of completed DVE ops; every op then_incs `dve`.
            # Same-engine RAW/WAR needs explicit self-waits (DVE write-back
            # is pipelined past the next op's read).
            cnt = [0]

            def emit(instr):
                instr.then_inc(dve, 1)
                cnt[0] += 1
                return cnt[0]

            emit(nc.vector.memset(z0t[:], z0))
            prev_q = 0  # count of the previous fold's q op (reads qa/qb)

            def fold(c):
                nonlocal prev_q
                vector.wait_ge(pdma[c % 3], 16 * (c // 3 + 1))
                if prev_q:
                    vector.wait_ge(dve, prev_q)  # WAR: qa/qb vs prev q read
                t = pb[c % 3]
                l = lambda i: t[:, i * MH:(i + 1) * MH]
                emit(nc.vector.tensor_add(qa[:], l(0), l(1)))
                nb = emit(nc.vector.tensor_add(qb[:], l(2), l(3)))
                vector.wait_ge(dve, nb)  # RAW qa,qb
                prev_q = emit(nc.vector.tensor_add(qs[c][:], qa[:], qb[:]))

            def reduce(k):
                vector.wait_ge(obdma[k % 3], 16 * (k // 3 + 1))
                emit(nc.vector.tensor_reduce(
                    ma[:, k * fc:(k + 1) * fc],
                    ob[k % 3][:].rearrange("p (f d) -> p f d", d=D),
                    axis=mybir.AxisListType.X,
                    op=mybir.AluOpType.max,
                    apply_absolute_value=True,
                ))

            def comb_ep(h):
                # qs for half h are chunks h*4 .. h*4+3 (see perf_view)
                q = qs[h * 4:h * 4 + 4]
                vector.wait_ge(dve, qdone[h * 4 + 3])  # RAW: all four q complete
                emit(nc.vector.tensor_add(sh[h][:], q[0][:], q[1][:]))
                n2 = emit(nc.vector.tensor_add(s2[h][:], q[2][:], q[3][:]))
                vector.wait_ge(dve, n2)
                n3 = emit(nc.vector.tensor_add(sh[h][:], sh[h][:], s2[h][:]))
                vector.wait_ge(dve, n3)
                n4 = emit(nc.vector.tensor_scalar(
                    dd[h][:], sh[h][:], neg_b, c_const,
                    op0=mybir.AluOpType.mult, op1=mybir.AluOpType.add,
                ))
                vector.wait_ge(dve, n4)
                emit(nc.vector.scalar_tensor_tensor(
                    xx[h][:], ma[:, h * MH:(h + 1) * MH], 0.0, dd[h][:],
                    op0=mybir.AluOpType.is_gt, op1=mybir.AluOpType.mult,
                ))

            for tok in order:
                if tok[0] == "f":
                    fold(int(tok[1]))
                elif tok[0] == "r":
                    reduce(int(tok[1]))
                elif tok[0] == "c":
                    comb_ep(int(tok[1]))
                # "e" ops are emitted inside comb_ep

            for h in range(2):
                vector.wait_ge(act, h + 1)
                emit(nc.vector.tensor_scalar(
                    oo[h][:], pp[h][:], 0.01, 0.99,
                    op0=mybir.AluOpType.max, op1=mybir.AluOpType.min,
                ))
            assert cnt[0] == clip1_n, (cnt[0], clip1_n)

        block_cm.__exit__(None, None, None)
        # Re-executable NEFF tail (the NTFF profiler replays it): one
        # all-engine barrier, then zero our semaphores.  Much cheaper than
        # TileContext's drain + double-barrier + per-proc reset tail.
        nc.all_engine_barrier()
        nc.gpsimd.dma_reset(sem_range)
        nc.gpsimd.sem_clear(sem_range)

    nc.compile()
    return nc


def _scalar_constants(inputs):
    t0 = float(np.asarray(inputs["trust0"]).reshape(()))
    s0 = float(np.asarray(inputs["sigma0"]).reshape(()))
    wb = float(np.asarray(inputs["wb"]).reshape(()))
    wtp = float(np.asarray(inputs["wtp"]).reshape(()))
    st = float(np.asarray(inputs["sigma_t"]).reshape(()))
    r1 = 1.0 / math.sqrt(s0 * s0 + T * st * st)
    z0 = t0 / math.sqrt(s0 * s0)
    a_const = (t0 + T * wb + T * wtp) * r1
    neg_b = -2.0 * wtp * r1
    c_const = a_const - z0
    return neg_b, c_const, z0


def run(inputs, trace=False, **kw):
    """Shard, run on 8 cores, gather. Returns (output [N,1] f32, exec_time_ns)."""
    from concourse.bass_utils import run_bass_kernel_spmd

    obs = np.asarray(inputs["inptasksobs"])
    perf = np.asarray(inputs["inptasksperf"])
    assert obs.shape == (T, N, D) and perf.shape == (T, N, 1)

    neg_b, c_const, z0 = _scalar_constants(inputs)
    nc = build_program(neg_b, c_const, z0)

    obs_p = np.zeros((NPAD, D), np.float32)
    obs_p[:N] = obs[0]
    perf_p = np.zeros((T, NPAD), np.float32)
    perf_p[:, :N] = perf[:, :, 0]

    in_maps = []
    for c in range(NCORES):
        oc = obs_p[c * PER:(c + 1) * PER].reshape(128, K, (F // K) * D)
        pc = np.ascontiguousarray(
            perf_p[:, c * PER:(c + 1) * PER]
        ).reshape(T, 128, F)
        in_maps.append({"obs0": oc, "perfc": pc})

    res = run_bass_kernel_spmd(
        nc, in_maps, core_ids=list(range(NCORES)), trace=trace, **kw
    )
    full = np.concatenate(
        [res.results[c]["out"].reshape(-1) for c in range(NCORES)]
    )
    return full[:N].reshape(N, 1).astype(np.float32, copy=False), res.exec_time_ns


def kernel(**inputs):
    out, _ = run(inputs, trace=False)
    return out
